# revision 35
# baseline (speedup 1.0000x reference)
"""Autoregressive GRU (B=128, H=1024, n_steps=512) on TRN2.

Key algebraic fact: the reference feeds each step's output back as the next
input, so x_t == h_t for every step after the first. The GRU cell then needs
only 4H of matmul columns per step:
    r = sigmoid(h @ (W_ir+W_hr).T + (b_ir+b_hr))
    z = sigmoid(h @ (W_iz+W_hz).T + (b_iz+b_hz))
    n = tanh(h @ W_in.T + b_in + r * (h @ W_hn.T + b_hn))
    h' = n + z*(h - n)
Step 1 (x0 != h0) is computed on the host; steps 2..511 run on one NeuronCore.

Device layout per step (single core):
  - state hT [H=1024 rows as 8 k-tiles, B=128] is the matmul stationary
    operand; combined weights WbigT [1024, 4096] stream as the moving operand
    (float32r: ~231 ns/matmul warm at N=512; full fp32 would be 4 passes).
  - 8 PSUM chunks [128, 512] = [r|z|hn|in] x 128 H-cols each; gates on
    DVE/ACT/GPSIMD; h_new chunk is PE-transposed back into the next hT
    (transposes emitted after all chunks so they never stall the PE FIFO).
"""

import os
import sys

sys.path.insert(0, "/opt/trn_rl_repo")

import numpy as np

B, H, T = 128, 1024, 512
NK = H // 128          # 8 contraction k-tiles
NCHUNK = 8             # output chunks of 512 cols (4*H / 512)
DEV_STEPS = T - 2      # steps computed on device (510), must be even

_cache = {}


def _build(dev_steps):
    from contextlib import ExitStack

    import concourse.bass as bass
    import concourse.tile as tile
    from concourse import bacc, mybir

    f32 = mybir.dt.float32
    if os.environ.get("GRU_BF16"):
        f32r = mybir.dt.bfloat16
        wdt = mybir.dt.bfloat16
    elif os.environ.get("GRU_WBF16"):
        f32r = mybir.dt.float32r
        wdt = mybir.dt.bfloat16
    else:
        f32r = mybir.dt.float32r
        wdt = f32r
    AF = mybir.ActivationFunctionType

    nc = bacc.Bacc("TRN2", target_bir_lowering=False, debug=False,
                   enable_asserts=False, num_devices=1)

    wct_d = nc.dram_tensor("wct", [H, 4 * H], wdt, kind="ExternalInput").ap()
    bias_d = nc.dram_tensor("bias", [B, 4 * H], f32, kind="ExternalInput").ap()
    h1_d = nc.dram_tensor("h1", [B, H], f32r, kind="ExternalInput").ap()
    h1t_d = nc.dram_tensor("h1t", [H, B], f32r, kind="ExternalInput").ap()
    ident_d = nc.dram_tensor("ident", [128, 128], f32r, kind="ExternalInput").ap()
    UNROLL = 30 if dev_steps % 30 == 0 else 2
    n_iter = dev_steps // UNROLL
    xs_d = nc.dram_tensor("xs", [B, n_iter, UNROLL, H], f32r,
                          kind="ExternalOutput").ap()

    with tile.TileContext(nc) as tc:
        with ExitStack() as ctx:
            const = ctx.enter_context(tc.tile_pool(name="const", bufs=1))
            state = ctx.enter_context(tc.tile_pool(name="state", bufs=1))
            tmp = ctx.enter_context(tc.tile_pool(name="tmp", bufs=5))
            psum = ctx.enter_context(
                tc.tile_pool(name="psum", bufs=5, space="PSUM"))
            psum_t = ctx.enter_context(
                tc.tile_pool(name="psum_t", bufs=3, space="PSUM"))

            wct = const.tile([128, NK * 4 * H], wdt)      # k-tile k at cols k*4H
            for k in range(NK):
                nc.sync.dma_start(wct[:, k * 4 * H:(k + 1) * 4 * H],
                                  wct_d[k * 128:(k + 1) * 128, :])
            bias = const.tile([128, 4 * H], f32)
            nc.sync.dma_start(bias[:], bias_d[:])
            ident = const.tile([128, 128], f32r)
            nc.sync.dma_start(ident[:], ident_d[:])

            hb = [state.tile([128, H], f32r, tag=f"h{i}", name=f"h{i}")
                  for i in range(2)]
            hTb = [[state.tile([128, 128], f32r, tag=f"ht{i}k{k}",
                                name=f"ht{i}k{k}") for k in range(NK)]
                   for i in range(2)]
            nc.sync.dma_start(hb[0][:], h1_d[:])
            for k in range(NK):
                nc.sync.dma_start(hTb[0][k][:],
                                  h1t_d[k * 128:(k + 1) * 128, :])

            def emit_tr(dst_h, dst_hT, c):
                cb = slice(c * 128, (c + 1) * 128)
                tps = psum_t.tile([128, 128], f32r, tag="tps")
                nc.tensor.transpose(tps[:], dst_h[:, cb], ident[:])
                nc.scalar.copy(dst_hT[c][:], tps[:])

            def step(src_h, src_hT, dst_h, dst_hT, t_iv, t_sub, pending):
                for c in range(NCHUNK):
                    co = c * 512
                    cb = slice(c * 128, (c + 1) * 128)
                    ps = psum.tile([128, 512], f32, tag="acc")
                    for k in range(NK):
                        if c == 0 and k == NK - 1 and pending is not None:
                            pending()
                        nc.tensor.matmul(
                            ps[:],
                            src_hT[k][:],
                            wct[:, k * 4 * H + co: k * 4 * H + co + 512],
                            start=(k == 0), stop=(k == NK - 1),
                        )
                    rzhnb = tmp.tile([128, 384], f32, tag="rzhnb")
                    nc.vector.tensor_add(rzhnb[:], ps[:, 0:384],
                                         bias[:, co:co + 384])
                    rz = tmp.tile([128, 256], f32, tag="rz")
                    nc.scalar.activation(rz[:], rzhnb[:, 0:256], AF.Sigmoid)
                    rhn = tmp.tile([128, 128], f32, tag="rhn")
                    nc.vector.tensor_mul(rhn[:], rz[:, 0:128], rzhnb[:, 256:384])
                    rhnb = tmp.tile([128, 128], f32, tag="rhnb")
                    nc.vector.tensor_add(rhnb[:], rhn[:], bias[:, co + 384:co + 512])
                    nin = tmp.tile([128, 128], f32, tag="nin")
                    nc.vector.tensor_add(nin[:], ps[:, 384:512], rhnb[:])
                    n = tmp.tile([128, 128], f32, tag="n")
                    nc.scalar.activation(n[:], nin[:], AF.Tanh)
                    hmn = tmp.tile([128, 128], f32, tag="hmn")
                    nc.gpsimd.tensor_sub(hmn[:], src_h[:, cb], n[:])
                    zhmn = tmp.tile([128, 128], f32, tag="zhmn")
                    nc.gpsimd.tensor_mul(zhmn[:], rz[:, 128:256], hmn[:])
                    nc.vector.tensor_add(dst_h[:, cb], n[:], zhmn[:])
                for c in range(NCHUNK - 1):
                    emit_tr(dst_h, dst_hT, c)
                nc.gpsimd.dma_start(xs_d[:, bass.ds(t_iv, 1), t_sub, :], dst_h[:])
                return lambda: emit_tr(dst_h, dst_hT, NCHUNK - 1)

            with tc.For_i(0, n_iter,
                          hint_engines=tuple(mybir.ALL_ENGINES)) as it:
                pending = None
                for u in range(UNROLL):
                    a, b = u % 2, 1 - (u % 2)
                    pending = step(hb[a], hTb[a], hb[b], hTb[b], it, u, pending)
                pending()   # flush the last transpose before the back-edge

    nc.compile()
    return nc


def _prep_inputs(x0, h0, W_ih, W_hh, b_ih, b_hh):
    x0 = np.asarray(x0, np.float32)
    h0 = np.asarray(h0, np.float32)
    W_ih = np.asarray(W_ih, np.float32)
    W_hh = np.asarray(W_hh, np.float32)
    b_ih = np.asarray(b_ih, np.float32)
    b_hh = np.asarray(b_hh, np.float32)

    # host step 1 (x0 != h0)
    x = x0[:, 0, :]
    h = h0[0]
    gi = x @ W_ih.T + b_ih
    gh = h @ W_hh.T + b_hh
    r = 1.0 / (1.0 + np.exp(-(gi[:, :H] + gh[:, :H])))
    z = 1.0 / (1.0 + np.exp(-(gi[:, H:2 * H] + gh[:, H:2 * H])))
    n = np.tanh(gi[:, 2 * H:] + r * gh[:, 2 * H:])
    h1 = ((1.0 - z) * n + z * h).astype(np.float32)

    # combined weights/biases in chunk-major layout
    Wr = W_ih[:H] + W_hh[:H]
    Wz = W_ih[H:2 * H] + W_hh[H:2 * H]
    Win = W_ih[2 * H:]
    Whn = W_hh[2 * H:]
    br = b_ih[:H] + b_hh[:H]
    bz = b_ih[H:2 * H] + b_hh[H:2 * H]
    bin_ = b_ih[2 * H:]
    bhn = b_hh[2 * H:]

    Wbig = np.empty((4 * H, H), np.float32)
    bvec = np.empty(4 * H, np.float32)
    for c in range(NCHUNK):
        jb = slice(c * 128, (c + 1) * 128)
        o = c * 512
        Wbig[o:o + 128] = Wr[jb]
        Wbig[o + 128:o + 256] = Wz[jb]
        Wbig[o + 256:o + 384] = Whn[jb]
        Wbig[o + 384:o + 512] = Win[jb]
        bvec[o:o + 128] = br[jb]
        bvec[o + 128:o + 256] = bz[jb]
        bvec[o + 256:o + 384] = bhn[jb]
        bvec[o + 384:o + 512] = bin_[jb]

    if os.environ.get("GRU_BF16"):
        import ml_dtypes
        mmdt = ml_dtypes.bfloat16
        wnp = mmdt
    elif os.environ.get("GRU_WBF16"):
        import ml_dtypes
        mmdt = np.float32
        wnp = ml_dtypes.bfloat16
    else:
        mmdt = np.float32
        wnp = np.float32
    in_map = {
        "wct": np.ascontiguousarray(Wbig.T).astype(wnp),
        "bias": np.ascontiguousarray(np.tile(bvec[None, :], (B, 1))),
        "h1": np.ascontiguousarray(h1),
        "h1t": np.ascontiguousarray(h1.T).astype(mmdt),
        "ident": np.eye(128, dtype=np.float32),
    }
    return in_map, x, h1


def kernel(x0, h0, n_steps, W_ih, W_hh, b_ih, b_hh, _dev_steps=None, _trace=False):
    from concourse.bass_utils import run_bass_kernel_spmd

    assert int(n_steps) == T, f"kernel hardcoded for n_steps={T}, got {n_steps}"
    dev_steps = _dev_steps if _dev_steps is not None else DEV_STEPS

    if dev_steps not in _cache:
        _cache[dev_steps] = _build(dev_steps)
    nc = _cache[dev_steps]

    in_map, x, h1 = _prep_inputs(x0, h0, W_ih, W_hh, b_ih, b_hh)
    res = run_bass_kernel_spmd(nc, [in_map], core_ids=[0], trace=_trace)
    dev_xs = res.results[0]["xs"].reshape(B, dev_steps, H)

    xs = np.empty((B, T, H), np.float32)
    xs[:, 0] = x
    xs[:, 1] = h1
    xs[:, 2:2 + dev_steps] = dev_xs
    if 2 + dev_steps < T:   # testing-only path (shortened device loop)
        xs = xs[:, :2 + dev_steps]
    h_final = np.ascontiguousarray(xs[:, -1])[None]
    kernel._last_result = res
    return xs, h_final


# revision 37
# speedup vs baseline: 1.0533x; 1.0533x over previous
"""Autoregressive GRU (B=128, H=1024, n_steps=512) on TRN2.

Key algebraic fact: the reference feeds each step's output back as the next
input, so x_t == h_t for every step after the first. The GRU cell then needs
only 4H of matmul columns per step:
    r = sigmoid(h @ (W_ir+W_hr).T + (b_ir+b_hr))
    z = sigmoid(h @ (W_iz+W_hz).T + (b_iz+b_hz))
    n = tanh(h @ W_in.T + b_in + r * (h @ W_hn.T + b_hn))
    h' = n + z*(h - n)
Step 1 (x0 != h0) is computed on the host; steps 2..511 run on one NeuronCore.

Device layout per step (single core):
  - state hT [H=1024 rows as 8 k-tiles, B=128] is the matmul stationary
    operand; combined weights WbigT [1024, 4096] stream as the moving operand
    (float32r: ~231 ns/matmul warm at N=512; full fp32 would be 4 passes).
  - 8 PSUM chunks [128, 512] = [r|z|hn|in] x 128 H-cols each; gates on
    DVE/ACT/GPSIMD; h_new chunk is PE-transposed back into the next hT
    (transposes emitted after all chunks so they never stall the PE FIFO).
"""

import os
import sys

sys.path.insert(0, "/opt/trn_rl_repo")

import numpy as np

B, H, T = 128, 1024, 512
NK = H // 128          # 8 contraction k-tiles
NCHUNK = 8             # output chunks of 512 cols (4*H / 512)
DEV_STEPS = T - 2      # steps computed on device (510), must be even

_cache = {}


def _build(dev_steps):
    from contextlib import ExitStack

    import concourse.bass as bass
    import concourse.tile as tile
    from concourse import bacc, mybir

    f32 = mybir.dt.float32
    if os.environ.get("GRU_BF16"):
        f32r = mybir.dt.bfloat16
        wdt = mybir.dt.bfloat16
    elif os.environ.get("GRU_WBF16"):
        f32r = mybir.dt.float32r
        wdt = mybir.dt.bfloat16
    else:
        f32r = mybir.dt.float32r
        wdt = f32r
    AF = mybir.ActivationFunctionType

    nc = bacc.Bacc("TRN2", target_bir_lowering=False, debug=False,
                   enable_asserts=False, num_devices=1)

    wct_d = nc.dram_tensor("wct", [H, 4 * H], wdt, kind="ExternalInput").ap()
    bias_d = nc.dram_tensor("bias", [B, 4 * H], f32, kind="ExternalInput").ap()
    h1_d = nc.dram_tensor("h1", [B, H], f32, kind="ExternalInput").ap()
    h1t_d = nc.dram_tensor("h1t", [H, B], f32r, kind="ExternalInput").ap()
    ident_d = nc.dram_tensor("ident", [128, 128], f32, kind="ExternalInput").ap()
    UNROLL = 34 if dev_steps % 34 == 0 else 2
    n_iter = dev_steps // UNROLL
    xs_d = nc.dram_tensor("xs", [B, n_iter, UNROLL, H], f32,
                          kind="ExternalOutput").ap()

    with tile.TileContext(nc) as tc:
        with ExitStack() as ctx:
            const = ctx.enter_context(tc.tile_pool(name="const", bufs=1))
            state = ctx.enter_context(tc.tile_pool(name="state", bufs=1))
            tmp = ctx.enter_context(tc.tile_pool(name="tmp", bufs=5))
            psum = ctx.enter_context(
                tc.tile_pool(name="psum", bufs=5, space="PSUM"))
            psum_t = ctx.enter_context(
                tc.tile_pool(name="psum_t", bufs=3, space="PSUM"))

            wct = const.tile([128, NK * 4 * H], wdt)      # k-tile k at cols k*4H
            for k in range(NK):
                nc.sync.dma_start(wct[:, k * 4 * H:(k + 1) * 4 * H],
                                  wct_d[k * 128:(k + 1) * 128, :])
            bias = const.tile([128, 4 * H], f32)
            nc.sync.dma_start(bias[:], bias_d[:])
            ident = const.tile([128, 128], f32)
            nc.sync.dma_start(ident[:], ident_d[:])

            hb = [state.tile([128, H], f32, tag=f"h{i}", name=f"h{i}")
                  for i in range(2)]
            hTb = [[state.tile([128, 128], f32r, tag=f"ht{i}k{k}",
                                name=f"ht{i}k{k}") for k in range(NK)]
                   for i in range(2)]
            nc.sync.dma_start(hb[0][:], h1_d[:])
            for k in range(NK):
                nc.sync.dma_start(hTb[0][k][:],
                                  h1t_d[k * 128:(k + 1) * 128, :])

            def emit_tr(dst_h, dst_hT, c):
                cb = slice(c * 128, (c + 1) * 128)
                tps = psum_t.tile([128, 128], f32, tag="tps")
                nc.tensor.transpose(tps[:], dst_h[:, cb], ident[:])
                nc.scalar.copy(dst_hT[c][:], tps[:])

            def step(src_h, src_hT, dst_h, dst_hT, t_iv, t_sub, pending):
                for c in range(NCHUNK):
                    co = c * 512
                    cb = slice(c * 128, (c + 1) * 128)
                    ps = psum.tile([128, 512], f32, tag="acc")
                    for k in range(NK):
                        if c == 0 and k == NK - 1 and pending is not None:
                            pending()
                        nc.tensor.matmul(
                            ps[:],
                            src_hT[k][:],
                            wct[:, k * 4 * H + co: k * 4 * H + co + 512],
                            start=(k == 0), stop=(k == NK - 1),
                        )
                    rzhnb = tmp.tile([128, 384], f32, tag="rzhnb")
                    nc.vector.tensor_add(rzhnb[:], ps[:, 0:384],
                                         bias[:, co:co + 384])
                    rz = tmp.tile([128, 256], f32, tag="rz")
                    nc.scalar.activation(rz[:], rzhnb[:, 0:256], AF.Sigmoid)
                    rhn = tmp.tile([128, 128], f32, tag="rhn")
                    nc.vector.tensor_mul(rhn[:], rz[:, 0:128], rzhnb[:, 256:384])
                    rhnb = tmp.tile([128, 128], f32, tag="rhnb")
                    nc.vector.tensor_add(rhnb[:], rhn[:], bias[:, co + 384:co + 512])
                    nin = tmp.tile([128, 128], f32, tag="nin")
                    nc.vector.tensor_add(nin[:], ps[:, 384:512], rhnb[:])
                    n = tmp.tile([128, 128], f32, tag="n")
                    nc.scalar.activation(n[:], nin[:], AF.Tanh)
                    hmn = tmp.tile([128, 128], f32, tag="hmn")
                    nc.gpsimd.tensor_sub(hmn[:], src_h[:, cb], n[:])
                    zhmn = tmp.tile([128, 128], f32, tag="zhmn")
                    nc.gpsimd.tensor_mul(zhmn[:], rz[:, 128:256], hmn[:])
                    nc.vector.tensor_add(dst_h[:, cb], n[:], zhmn[:])
                for c in range(NCHUNK - 1):
                    emit_tr(dst_h, dst_hT, c)
                nc.gpsimd.dma_start(xs_d[:, bass.ds(t_iv, 1), t_sub, :], dst_h[:])
                return lambda: emit_tr(dst_h, dst_hT, NCHUNK - 1)

            with tc.For_i(0, n_iter,
                          hint_engines=tuple(mybir.ALL_ENGINES)) as it:
                pending = None
                for u in range(UNROLL):
                    a, b = u % 2, 1 - (u % 2)
                    pending = step(hb[a], hTb[a], hb[b], hTb[b], it, u, pending)
                pending()   # flush the last transpose before the back-edge

    nc.compile()
    return nc


def _prep_inputs(x0, h0, W_ih, W_hh, b_ih, b_hh):
    x0 = np.asarray(x0, np.float32)
    h0 = np.asarray(h0, np.float32)
    W_ih = np.asarray(W_ih, np.float32)
    W_hh = np.asarray(W_hh, np.float32)
    b_ih = np.asarray(b_ih, np.float32)
    b_hh = np.asarray(b_hh, np.float32)

    # host step 1 (x0 != h0)
    x = x0[:, 0, :]
    h = h0[0]
    gi = x @ W_ih.T + b_ih
    gh = h @ W_hh.T + b_hh
    r = 1.0 / (1.0 + np.exp(-(gi[:, :H] + gh[:, :H])))
    z = 1.0 / (1.0 + np.exp(-(gi[:, H:2 * H] + gh[:, H:2 * H])))
    n = np.tanh(gi[:, 2 * H:] + r * gh[:, 2 * H:])
    h1 = ((1.0 - z) * n + z * h).astype(np.float32)

    # combined weights/biases in chunk-major layout
    Wr = W_ih[:H] + W_hh[:H]
    Wz = W_ih[H:2 * H] + W_hh[H:2 * H]
    Win = W_ih[2 * H:]
    Whn = W_hh[2 * H:]
    br = b_ih[:H] + b_hh[:H]
    bz = b_ih[H:2 * H] + b_hh[H:2 * H]
    bin_ = b_ih[2 * H:]
    bhn = b_hh[2 * H:]

    Wbig = np.empty((4 * H, H), np.float32)
    bvec = np.empty(4 * H, np.float32)
    for c in range(NCHUNK):
        jb = slice(c * 128, (c + 1) * 128)
        o = c * 512
        Wbig[o:o + 128] = Wr[jb]
        Wbig[o + 128:o + 256] = Wz[jb]
        Wbig[o + 256:o + 384] = Whn[jb]
        Wbig[o + 384:o + 512] = Win[jb]
        bvec[o:o + 128] = br[jb]
        bvec[o + 128:o + 256] = bz[jb]
        bvec[o + 256:o + 384] = bhn[jb]
        bvec[o + 384:o + 512] = bin_[jb]

    if os.environ.get("GRU_BF16"):
        import ml_dtypes
        mmdt = ml_dtypes.bfloat16
        wnp = mmdt
    elif os.environ.get("GRU_WBF16"):
        import ml_dtypes
        mmdt = np.float32
        wnp = ml_dtypes.bfloat16
    else:
        mmdt = np.float32
        wnp = np.float32
    in_map = {
        "wct": np.ascontiguousarray(Wbig.T).astype(wnp),
        "bias": np.ascontiguousarray(np.tile(bvec[None, :], (B, 1))),
        "h1": np.ascontiguousarray(h1),
        "h1t": np.ascontiguousarray(h1.T).astype(mmdt),
        "ident": np.eye(128, dtype=np.float32),
    }
    return in_map, x, h1


def kernel(x0, h0, n_steps, W_ih, W_hh, b_ih, b_hh, _dev_steps=None, _trace=False):
    from concourse.bass_utils import run_bass_kernel_spmd

    assert int(n_steps) == T, f"kernel hardcoded for n_steps={T}, got {n_steps}"
    dev_steps = _dev_steps if _dev_steps is not None else DEV_STEPS

    if dev_steps not in _cache:
        _cache[dev_steps] = _build(dev_steps)
    nc = _cache[dev_steps]

    in_map, x, h1 = _prep_inputs(x0, h0, W_ih, W_hh, b_ih, b_hh)
    res = run_bass_kernel_spmd(nc, [in_map], core_ids=[0], trace=_trace)
    dev_xs = res.results[0]["xs"].reshape(B, dev_steps, H)

    xs = np.empty((B, T, H), np.float32)
    xs[:, 0] = x
    xs[:, 1] = h1
    xs[:, 2:2 + dev_steps] = dev_xs
    if 2 + dev_steps < T:   # testing-only path (shortened device loop)
        xs = xs[:, :2 + dev_steps]
    h_final = np.ascontiguousarray(xs[:, -1])[None]
    kernel._last_result = res
    return xs, h_final
